# revision 9
# baseline (speedup 1.0000x reference)
"""Trainium2 Bass kernel for nn_CosmosPatcher3d.

Computes the Cosmos 3D Haar wavelet patcher: input [1,3,33,704,704] fp32,
temporal causal pad (first frame repeated 4x -> 36 frames), then two full
3D Haar DWT levels. Equivalent to a separable +-1 Hadamard transform over
4x4x4 blocks scaled by 1/64, producing [1,192,9,176,176] fp32 with channel
layout ch = 96*T2 + 48*H2 + 24*W2 + 12*T1 + 6*H1 + 3*W1 + c.

Strategy (8 NeuronCores, shard along H: 704 = 8*88, 4-blocks don't straddle):
- Host splits x into bf16 hi/lo (x ~= hi + lo to ~2^-18 relative) so TensorE
  runs at bf16 rate while accumulating both halves into fp32 PSUM: exact to
  ~1e-5. Same HBM bytes as fp32.
- TensorE: fused T+H transform. lhsT = +-1/64 sign matrix [K=(dt,hh),
  M=(TH2, y', TH1)], rhs = [K, (hi|lo, w)] per channel.
- ScalarE: PSUM -> SBUF copies. VectorE: two-level strided W butterfly,
  output free layout (W2, W1, c, x').
- Stores: per (TH2, W2) -> HBM AP [y', (TH1,W1,c) merged 24 @ ch-stride, x]
  = 3 dims, 8 stores x 135KB per (t, chunk).
"""

import ml_dtypes
import numpy as np

import concourse.bacc as bacc
import concourse.mybir as mybir
import concourse.tile as tile
from concourse.bass_utils import run_bass_kernel_spmd

N_CORES = 8
C = 3            # input channels
T_IN = 33        # input frames
H_IN = 704       # input height (global)
W_IN = 704       # input width
H_SH = H_IN // N_CORES      # 88 input rows per core
T_OUT = 9
Y_SH = H_SH // 4            # 22 output rows per core
X_OUT = W_IN // 4           # 176
FREE = C * W_IN             # 2112
CHUNKS = [(0, 32), (32, 32), (64, 24)]

_F32 = mybir.dt.float32
_BF16 = mybir.dt.bfloat16
_BF16_NP = ml_dtypes.bfloat16


def _sgn1d(pos, b2, b1):
    """Composite 2-level Haar sign for position pos in 0..3 (+-1)."""
    s1 = 1.0 if b1 == 0 else (1.0 - 2.0 * (pos % 2))
    s2 = 1.0 if b2 == 0 else (1.0 - 2.0 * (pos // 2))
    return s1 * s2


def _build_signs():
    """bf16 sign matrices including the global 1/64 scale (exact in bf16).

    M ordering: m = (T2*2+H2)*32 + y'*4 + (T1*2+H1)   (y' = hh//4).
    s32 [128,128]: rows dt*32+hh, t>=1 chunks of 32 rows.
    s24 [96,128]:  rows dt*24+hh, 24-row chunk (y'>5 columns zero).
    t32 [32,128] / t24 [24,128]: t=0 (frame 0 repeated 4x -> only T2=T1=0
    subbands nonzero, weight 4).
    """
    def mk(nh, t0):
        k = nh if t0 else 4 * nh
        s = np.zeros((k, 128), dtype=np.float32)
        for hh in range(nh):
            yp, hp = hh // 4, hh % 4
            for t2 in range(2):
                for h2 in range(2):
                    for t1 in range(2):
                        for h1 in range(2):
                            col = (t2 * 2 + h2) * 32 + yp * 4 + (t1 * 2 + h1)
                            sh = _sgn1d(hp, h2, h1)
                            if t0:
                                if t2 == 0 and t1 == 0:
                                    s[hh, col] = 4.0 * sh / 64.0
                            else:
                                for dt in range(4):
                                    st = _sgn1d(dt, t2, t1)
                                    s[dt * nh + hh, col] = st * sh / 64.0
        return s.astype(_BF16_NP)

    return mk(32, False), mk(24, False), mk(32, True), mk(24, True)


def _build_nc():
    nc = bacc.Bacc(
        "TRN2", target_bir_lowering=False, debug=False, num_devices=N_CORES
    )
    # x split hi/lo: [C, T, H, 2, W] bf16
    x = nc.dram_tensor(
        "x", [C, T_IN, H_SH, 2, W_IN], _BF16, kind="ExternalInput"
    ).ap()
    s32 = nc.dram_tensor("s32", [128, 128], _BF16, kind="ExternalInput").ap()
    s24 = nc.dram_tensor("s24", [96, 128], _BF16, kind="ExternalInput").ap()
    t32 = nc.dram_tensor("t32", [32, 128], _BF16, kind="ExternalInput").ap()
    t24 = nc.dram_tensor("t24", [24, 128], _BF16, kind="ExternalInput").ap()
    out = nc.dram_tensor(
        "out", [192, T_OUT, Y_SH, X_OUT], _F32, kind="ExternalOutput"
    ).ap()

    # Store view: ch = 48*TH2 + 24*W2 + (6*TH1 + 3*W1 + c).
    # Per (TH2, W2) store: HBM dims [y', i(24) @ ch-stride, x] -> 3 dims.
    o_v = out.rearrange("(a b i) t y x -> a b t y i x", a=4, b=2, i=24)

    with tile.TileContext(nc) as tc:
        with (
            tc.tile_pool(name="signs", bufs=1) as sgp,
            tc.tile_pool(name="rhs", bufs=4) as rhp,
            tc.tile_pool(name="sbf", bufs=3) as fbp,
            tc.tile_pool(name="sbsd", bufs=3) as sdp,
            tc.tile_pool(name="outp", bufs=4) as otp,
            tc.tile_pool(name="psum", bufs=4, space="PSUM") as psp,
        ):
            ts32 = sgp.tile([128, 128], _BF16)
            ts24 = sgp.tile([96, 128], _BF16)
            tt32 = sgp.tile([32, 128], _BF16)
            tt24 = sgp.tile([24, 128], _BF16)
            nc.sync.dma_start(out=ts32, in_=s32)
            nc.sync.dma_start(out=ts24, in_=s24)
            nc.sync.dma_start(out=tt32, in_=t32)
            nc.sync.dma_start(out=tt24, in_=t24)

            store_i = 0
            for t in range(T_OUT):
                for ci, (h0, nh) in enumerate(CHUNKS):
                    ny = nh // 4
                    kdim = nh if t == 0 else 4 * nh
                    if t == 0:
                        lhsT = tt32 if nh == 32 else tt24
                    else:
                        lhsT = ts32 if nh == 32 else ts24

                    # rhs free layout per c: (hl(2), w)
                    rhs = rhp.tile([128, C, 2 * W_IN], _BF16, tag="rhs")
                    sbf = fbp.tile([128, FREE], _F32, tag="sbf")
                    for c in range(C):
                        if t == 0:
                            src = x[c, 0, h0 : h0 + nh, :, :]
                        else:
                            src = x[c, 4 * t - 3 : 4 * t + 1, h0 : h0 + nh, :, :]
                        nc.sync.dma_start(out=rhs[:kdim, c, :], in_=src)
                        ps = psp.tile([128, W_IN], _F32, tag="ps")
                        for j in range(0, W_IN, 512):
                            n = min(512, W_IN - j)
                            nc.tensor.matmul(
                                ps[:, j : j + n],
                                lhsT,
                                rhs[:kdim, c, j : j + n],
                                start=True,
                                stop=False,
                            )
                            nc.tensor.matmul(
                                ps[:, j : j + n],
                                lhsT,
                                rhs[:kdim, c, W_IN + j : W_IN + j + n],
                                start=False,
                                stop=True,
                            )
                        nc.scalar.copy(
                            out=sbf[:, c * W_IN : (c + 1) * W_IN],
                            in_=ps,
                        )

                    # W level 1: pairs along w -> sums (W1=0) and diffs (W1=1)
                    sbsd = sdp.tile([128, FREE], _F32, tag="sbsd")
                    v = sbf.rearrange("q (c w par) -> q c w par", c=C, par=2)
                    s_half = sbsd[:, : FREE // 2].rearrange("q (c w) -> q c w", c=C)
                    d_half = sbsd[:, FREE // 2 :].rearrange("q (c w) -> q c w", c=C)
                    nc.vector.tensor_add(
                        out=s_half, in0=v[:, :, :, 0], in1=v[:, :, :, 1]
                    )
                    nc.vector.tensor_sub(
                        out=d_half, in0=v[:, :, :, 0], in1=v[:, :, :, 1]
                    )

                    # W level 2 -> out tile free = (W2, W1, c, x')
                    ot = otp.tile([128, FREE], _F32, tag="ot")
                    ov = ot.rearrange(
                        "q (W2 W1 c xx) -> q W2 W1 c xx", W2=2, W1=2, c=C
                    )
                    vs = sbsd[:, : FREE // 2].rearrange(
                        "q (c xx par) -> q c xx par", c=C, par=2
                    )
                    vd = sbsd[:, FREE // 2 :].rearrange(
                        "q (c xx par) -> q c xx par", c=C, par=2
                    )
                    nc.vector.tensor_add(
                        out=ov[:, 0, 0], in0=vs[:, :, :, 0], in1=vs[:, :, :, 1]
                    )
                    nc.vector.tensor_sub(
                        out=ov[:, 1, 0], in0=vs[:, :, :, 0], in1=vs[:, :, :, 1]
                    )
                    nc.vector.tensor_add(
                        out=ov[:, 0, 1], in0=vd[:, :, :, 0], in1=vd[:, :, :, 1]
                    )
                    nc.vector.tensor_sub(
                        out=ov[:, 1, 1], in0=vd[:, :, :, 0], in1=vd[:, :, :, 1]
                    )

                    # stores: one per (TH2, W2); partitions th2*32 + y'*4 + th1
                    y0 = h0 // 4
                    for th2 in range(4):
                        for w2 in range(2):
                            dst = o_v[th2, w2, t, y0 : y0 + ny]
                            # weighted split: 3 sync / 2 scalar / 4 gpsimd per 9
                            r = store_i % 9
                            eng = (
                                nc.sync
                                if r < 3
                                else (nc.scalar if r < 5 else nc.gpsimd)
                            )
                            store_i += 1
                            eng.dma_start(
                                out=dst,
                                in_=ot[
                                    th2 * 32 : th2 * 32 + 4 * ny,
                                    w2 * 1056 : (w2 + 1) * 1056,
                                ],
                            )

    nc.compile()
    return nc


_NC_CACHE = None


def _prep_inputs(hs):
    """Shard along H and split into bf16 hi/lo, interleaved as [..., 2, W]."""
    s32, s24, t32, t24 = _build_signs()
    in_maps = []
    for k in range(N_CORES):
        xk = np.ascontiguousarray(hs[0, :, :, k * H_SH : (k + 1) * H_SH, :])
        hi = xk.astype(_BF16_NP)
        lo = (xk - hi.astype(np.float32)).astype(_BF16_NP)
        xhl = np.stack([hi, lo], axis=3)  # [C, T, H, 2, W]
        in_maps.append(
            {"x": xhl, "s32": s32, "s24": s24, "t32": t32, "t24": t24}
        )
    return in_maps


def kernel(hidden_states: np.ndarray) -> np.ndarray:
    global _NC_CACHE
    if _NC_CACHE is None:
        _NC_CACHE = _build_nc()
    nc = _NC_CACHE

    hs = np.asarray(hidden_states, dtype=np.float32)
    assert hs.shape == (1, C, T_IN, H_IN, W_IN), hs.shape
    in_maps = _prep_inputs(hs)

    res = run_bass_kernel_spmd(nc, in_maps, core_ids=list(range(N_CORES)))

    out = np.empty((1, 192, T_OUT, H_IN // 4, X_OUT), dtype=np.float32)
    for k in range(N_CORES):
        out[0, :, :, k * Y_SH : (k + 1) * Y_SH, :] = res.results[k]["out"]
    return out
